# revision 17
# baseline (speedup 1.0000x reference)
"""Trainium2 Bass kernel for nn_BetweennessModule.

Math: content = x @ W.T + b; d1[i] = |content[i+1]-content[i]|,
d2[i] = |content[i+2]-content[i]|. The bias cancels in every difference, so
with u[i] = (x[i+1]-x[i]) @ W.T:
    d1[i]^2 = |u[i]|^2 =: s1[i]
    d2[i]^2 = |u[i]+u[i+1]|^2 = s1[i] + s1[i+1] + 2*(u[i].u[i+1]) =: s2[i]
score[i] = relu(1 - (d1[i]+d1[i+1]-d2[i]) / max(d2[i], eps))
adj[s]   = gate*0.5*0.1 * (score[s-1]/(S-2) - 0.5)   (score term 0 at s=0, S-1)

Sharding: pure data parallel, batch b -> core b. W/gate replicated. x shards
are fed pre-transposed ([D, S], a host-side layout choice) so the contraction
dim d lands on SBUF partitions with no on-chip transpose.
"""

import sys

sys.path.insert(0, "/opt/trn_rl_repo")

import numpy as np

import concourse.bass as bass
import concourse.mybir as mybir
import concourse.tile as tile
from concourse import bacc
from concourse.bass_utils import run_bass_kernel_spmd
from concourse.masks import make_identity

F32 = mybir.dt.float32
BF16 = mybir.dt.bfloat16
AF = mybir.ActivationFunctionType
ALU = mybir.AluOpType

B, S, D = 8, 4096, 1024
NK = D // 128  # 8 contraction tiles
NBLK = S // 128  # 32 sequence blocks of 128
CHUNK = 512  # s-columns per streamed chunk
NCHUNK = S // CHUNK  # 8
EPS = 1e-6
ADJ_SCALE = 0.1


def build_nc():
    nc = bacc.Bacc("TRN2", target_bir_lowering=False, debug=False)

    xT = nc.dram_tensor("xT", [D, S], F32, kind="ExternalInput")
    WT = nc.dram_tensor("WT", [D, D], F32, kind="ExternalInput")
    gate = nc.dram_tensor("gate", [1], F32, kind="ExternalInput")
    out = nc.dram_tensor("out", [S], F32, kind="ExternalOutput")

    with tile.TileContext(nc) as tc:
        with (
            tc.tile_pool(name="wt", bufs=1) as wt_pool,
            tc.tile_pool(name="persist", bufs=1) as persist,
            tc.tile_pool(name="xc", bufs=2) as xc_pool,
            tc.tile_pool(name="dxc", bufs=2) as dxc_pool,
            tc.tile_pool(name="scratch", bufs=2) as scratch,
            tc.tile_pool(name="us", bufs=3) as us_pool,
            tc.tile_pool(name="ush", bufs=3) as ush_pool,
            tc.tile_pool(name="psum", bufs=3, space="PSUM") as psum_pool,
            tc.tile_pool(name="psum_misc", bufs=1, space="PSUM") as psum_misc,
        ):
            # ---- resident weights W.T, [d, e] layout, 8 partition chunks
            wt = []
            for k in range(NK):
                t = wt_pool.tile([128, D], BF16, tag=f"wt{k}")
                nc.gpsimd.dma_start(t[:], WT[k * 128 : (k + 1) * 128, :])
                wt.append(t)

            # ---- gate broadcast to [32, 1] via a tiny K=1 matmul
            g_sb = persist.tile([1, 1], F32, tag="g_sb")
            nc.sync.dma_start(g_sb[:], gate[:].rearrange("(a b) -> a b", a=1))
            ones32 = persist.tile([1, 32], F32, tag="ones32")
            nc.vector.memset(ones32[:], 1.0)
            g_ps = psum_misc.tile([32, 1], F32, tag="g_ps")
            nc.tensor.matmul(g_ps[:], lhsT=ones32[:], rhs=g_sb[:], start=True, stop=True)
            g32 = persist.tile([32, 1], F32, tag="g32")
            nc.scalar.activation(g32[:], g_ps[:], AF.Copy)
            a_col = persist.tile([32, 1], F32, tag="a_col")
            nc.scalar.mul(a_col[:], g32[:], 0.5 * ADJ_SCALE / (S - 2))
            b_col = persist.tile([32, 1], F32, tag="b_col")
            nc.scalar.mul(b_col[:], g32[:], -0.5 * ADJ_SCALE * 0.5)

            # ---- stats accumulators: s1 in cols [0,32), c in cols [32,64)
            stats = persist.tile([128, 64], F32, tag="stats")
            zrow = persist.tile([1, D], BF16, tag="zrow")
            nc.vector.memset(zrow[:], 0.0)

            # ---- main loop: stream xT, diff, matmul, fused reductions.
            # Partition-base rule (walrus): compute-engine APs must start at
            # partition 0/32/64/96 — in SBUF *and* PSUM. The u[i]*u[i+1]
            # cross-term therefore uses a DMA (partition-unrestricted) to build
            # a one-row-shifted bf16 copy of each U block, and a base-0 DVE
            # tensor_tensor_reduce against it.
            BPC = CHUNK // 128  # blocks per chunk (4)
            CW = CHUNK + 1  # loaded columns per chunk (1-col lookahead)
            prev = None  # (us_c, ush_c, c) of the previous chunk

            def emit_cross(us_c, ush_c, ci):
                # c[i] = sum_e u[i,e]*u[i+1,e] for the whole chunk: one DVE mul
                # + one 3D-AP reduce producing 4 stats columns.
                # (tensor_tensor_reduce / accum_out on DVE crash the exec unit
                # in this runtime; plain mul + tensor_reduce are fine.)
                cs = scratch.tile([128, BPC * D], BF16, tag="cs")
                nc.gpsimd.tensor_mul(cs[:], us_c[:], ush_c[:])
                nc.vector.tensor_reduce(
                    stats[:, 32 + BPC * ci : 32 + BPC * (ci + 1)],
                    cs[:].rearrange("p (m e) -> p m e", e=D),
                    axis=mybir.AxisListType.X,
                    op=ALU.add,
                )

            for c in range(NCHUNK):
                last_chunk = c == NCHUNK - 1
                ncols = CHUNK if last_chunk else CW
                # one 2.1MB DMA per chunk: [128, 8, ncols] 3D access pattern
                xc = xc_pool.tile([128, NK * CW], F32, tag="xc")
                nc.sync.dma_start(
                    xc[:].rearrange("p (k j) -> p k j", k=NK)[:, :, 0:ncols],
                    xT[:, c * CHUNK : c * CHUNK + ncols].rearrange(
                        "(k p) j -> p k j", p=128
                    ),
                )
                # dx in bf16: one 3D DVE subtract per block (so block m's
                # matmuls never wait on later columns)
                dxc = dxc_pool.tile([128, NK * CHUNK], BF16, tag="dxc")
                x3 = xc[:].rearrange("p (k j) -> p k j", k=NK)
                d3 = dxc[:].rearrange("p (k j) -> p k j", k=NK)

                us_c = us_pool.tile([128, BPC * D], BF16, tag="us")
                ush_c = ush_pool.tile([128, BPC * D], BF16, tag="ush")
                for m in range(BPC):
                    g = c * BPC + m
                    lo = m * 128
                    hi = (m + 1) * 128
                    nd = hi - 1 if (last_chunk and m == BPC - 1) else hi
                    nc.vector.tensor_sub(
                        d3[:, :, lo:nd], x3[:, :, lo + 1 : nd + 1], x3[:, :, lo:nd]
                    )
                    if nd < hi:
                        nc.gpsimd.memset(d3[:, :, nd:hi], 0.0)
                    U = psum_pool.tile([128, D], F32, tag="U")
                    for n in range(2):
                        for k in range(NK):
                            nc.tensor.matmul(
                                U[:, n * 512 : (n + 1) * 512],
                                lhsT=dxc[:, k * CHUNK + lo : k * CHUNK + hi],
                                rhs=wt[k][:, n * 512 : (n + 1) * 512],
                                start=(k == 0),
                                stop=(k == NK - 1),
                            )
                    # s1[g*128+i] = sum_e U[i,e]^2  (ACT: square + row-accum)
                    sq = scratch.tile([128, D], F32, tag="sq")
                    nc.scalar.activation(
                        sq[:], U[:], AF.Square, accum_out=stats[:, g : g + 1]
                    )
                    # bf16 copy of U into the chunk-level buffer
                    nc.scalar.activation(us_c[:, m * D : (m + 1) * D], U[:], AF.Copy)

                # row-shifted copy via SWDGE DMAs (partition-unrestricted):
                # ush_c[i, m*D+e] = u[128*(4c+m) + i + 1, e]
                nc.gpsimd.dma_start(ush_c[0:127, :], us_c[1:128, :])
                nc.gpsimd.dma_start(
                    ush_c[127:128, 0 : (BPC - 1) * D], us_c[0:1, D : BPC * D]
                )
                if prev is not None:
                    pus_c, push_c, pc_ = prev
                    nc.gpsimd.dma_start(
                        push_c[127:128, (BPC - 1) * D : BPC * D], us_c[0:1, 0:D]
                    )
                    emit_cross(pus_c, push_c, pc_)
                prev = (us_c, ush_c, c)
            # final chunk: u[4096] does not exist -> zero row, c[4095] unused
            pus_c, push_c, pc_ = prev
            nc.gpsimd.dma_start(push_c[127:128, (BPC - 1) * D : BPC * D], zrow[:])
            emit_cross(pus_c, push_c, pc_)

            # ---- transpose stats [128, 64] -> [64, 128]: rows 0..31 = s1_t,
            #      rows 32..63 = c_t, column j = within-block index i
            ident = persist.tile([128, 128], F32, tag="ident")
            make_identity(nc, ident[:])
            st_ps = psum_misc.tile([64, 128], F32, tag="st_ps")
            nc.tensor.transpose(st_ps[:], stats[:], ident[:])
            s1_t = persist.tile([32, 128], F32, tag="s1_t")
            nc.scalar.activation(s1_t[:], st_ps[0:32, :], AF.Copy)
            c_t = persist.tile([32, 128], F32, tag="c_t")
            nc.scalar.activation(c_t[:], st_ps[32:64, :], AF.Copy)

            # ---- s1 shifted by one flat position: s1n[m, j] = s1[128m + j + 1]
            # main part is a free-dim shift; seam column 127 needs s1[128(m+1)]
            # = stats[0, m+1], partition-scattered via a tiny DMA.
            s1n = persist.tile([32, 128], F32, tag="s1n")
            nc.vector.tensor_copy(s1n[:, 0:127], s1_t[:, 1:128])
            row32 = persist.tile([1, 32], F32, tag="row32")
            nc.vector.tensor_copy(row32[0:1, 0:31], stats[0:1, 1:32])
            nc.vector.memset(row32[0:1, 31:32], 0.0)
            nc.sync.dma_start(s1n[0:32, 127:128], row32[0:1, 0:32])

            # s2 = s1 + s1n + 2c
            s2_t = persist.tile([32, 128], F32, tag="s2_t")
            nc.vector.tensor_add(s2_t[:], s1_t[:], s1n[:])
            c2_t = persist.tile([32, 128], F32, tag="c2_t")
            nc.vector.tensor_scalar_mul(c2_t[:], c_t[:], 2.0)
            nc.vector.tensor_add(s2_t[:], s2_t[:], c2_t[:])

            # d1[i], d1[i+1], d2[i]
            d1_t = persist.tile([32, 128], F32, tag="d1_t")
            nc.scalar.activation(d1_t[:], s1_t[:], AF.Sqrt)
            d1n = persist.tile([32, 128], F32, tag="d1n")
            nc.scalar.activation(d1n[:], s1n[:], AF.Sqrt)
            d2_t = persist.tile([32, 128], F32, tag="d2_t")
            nc.scalar.activation(d2_t[:], s2_t[:], AF.Sqrt)

            # path[i] = d1[i] + d1[i+1] (no seams: both operands flat-aligned)
            path = persist.tile([32, 128], F32, tag="path")
            nc.vector.tensor_add(path[:], d1_t[:], d1n[:])

            # score = relu(1 - (path - d2) / max(d2, eps))
            denom = persist.tile([32, 128], F32, tag="denom")
            nc.vector.tensor_scalar_max(denom[:], d2_t[:], EPS)
            rec = persist.tile([32, 128], F32, tag="rec")
            nc.vector.reciprocal(rec[:], denom[:])
            num = persist.tile([32, 128], F32, tag="num")
            nc.vector.tensor_sub(num[:], path[:], d2_t[:])
            ratio = persist.tile([32, 128], F32, tag="ratio")
            nc.vector.tensor_mul(ratio[:], num[:], rec[:])
            score = persist.tile([32, 128], F32, tag="score")
            nc.scalar.activation(score[:], ratio[:], AF.Relu, scale=-1.0, bias=1.0)

            # adj[i] = a*score[i] + b, shipped to out[i+1] via DMA addressing;
            # boundary cells out[0], out[4095] get the bare b value.
            adj_t = persist.tile([32, 128], F32, tag="adj_t")
            nc.vector.tensor_scalar(
                out=adj_t[:],
                in0=score[:],
                scalar1=a_col[:],
                scalar2=b_col[:],
                op0=ALU.mult,
                op1=ALU.add,
            )
            bb = persist.tile([1, 2], F32, tag="bb")
            nc.scalar.activation(bb[0:1, 0:1], b_col[0:1, :], AF.Copy)
            nc.scalar.activation(bb[0:1, 1:2], b_col[0:1, :], AF.Copy)

            # out[1 : 3969] <- adj flat [0 : 3968)
            nc.sync.dma_start(
                out[1:3969].rearrange("(p f) -> p f", f=128), adj_t[0:31, :]
            )
            # out[3969 : 4095] <- adj flat [3968 : 4094)
            nc.sync.dma_start(
                out[3969:4095].rearrange("(p f) -> p f", p=1), adj_t[31:32, 0:126]
            )
            nc.sync.dma_start(out[0:1].rearrange("(p f) -> p f", p=1), bb[0:1, 0:1])
            nc.sync.dma_start(out[4095:4096].rearrange("(p f) -> p f", p=1), bb[0:1, 1:2])

    nc.compile()
    return nc


_NC_CACHE = None


def kernel(x, W, b, gate):
    global _NC_CACHE
    x = np.asarray(x, dtype=np.float32)
    W = np.asarray(W, dtype=np.float32)
    gate = np.asarray(gate, dtype=np.float32)

    if _NC_CACHE is None:
        _NC_CACHE = build_nc()
    nc = _NC_CACHE

    WT_np = np.ascontiguousarray(W.T)
    in_maps = [
        {
            "xT": np.ascontiguousarray(x[i].T),
            "WT": WT_np,
            "gate": gate,
        }
        for i in range(B)
    ]
    res = run_bass_kernel_spmd(nc, in_maps, core_ids=list(range(B)))
    return np.stack([res.results[i]["out"] for i in range(B)]).astype(np.float32)


if __name__ == "__main__":
    # quick smoke: build only
    nc = build_nc()
    print("built ok")


# revision 18
# speedup vs baseline: 1.7843x; 1.7843x over previous
"""Trainium2 Bass kernel for nn_BetweennessModule.

Math: content = x @ W.T + b; d1[i] = |content[i+1]-content[i]|,
d2[i] = |content[i+2]-content[i]|. The bias cancels in every difference, so
with u[i] = (x[i+1]-x[i]) @ W.T:
    d1[i]^2 = |u[i]|^2 =: s1[i]
    d2[i]^2 = |u[i]+u[i+1]|^2 = s1[i] + s1[i+1] + 2*(u[i].u[i+1]) =: s2[i]
score[i] = relu(1 - (d1[i]+d1[i+1]-d2[i]) / max(d2[i], eps))
adj[s]   = gate*0.5*0.1 * (score[s-1]/(S-2) - 0.5)   (score term 0 at s=0, S-1)

Sharding: pure data parallel, batch b -> core b. W/gate replicated. x shards
are fed pre-transposed ([D, S], a host-side layout choice) so the contraction
dim d lands on SBUF partitions with no on-chip transpose.
"""

import sys

sys.path.insert(0, "/opt/trn_rl_repo")

import numpy as np

import concourse.bass as bass
import concourse.mybir as mybir
import concourse.tile as tile
from concourse import bacc
from concourse.bass_utils import run_bass_kernel_spmd
from concourse.masks import make_identity

F32 = mybir.dt.float32
BF16 = mybir.dt.bfloat16
AF = mybir.ActivationFunctionType
ALU = mybir.AluOpType

B, S, D = 8, 4096, 1024
NK = D // 128  # 8 contraction tiles
NBLK = S // 128  # 32 sequence blocks of 128
CHUNK = 512  # s-columns per streamed chunk
NCHUNK = S // CHUNK  # 8
EPS = 1e-6
ADJ_SCALE = 0.1


def build_nc():
    nc = bacc.Bacc("TRN2", target_bir_lowering=False, debug=False)

    xT = nc.dram_tensor("xT", [D, S], F32, kind="ExternalInput")
    WT = nc.dram_tensor("WT", [D, D], F32, kind="ExternalInput")
    gate = nc.dram_tensor("gate", [1], F32, kind="ExternalInput")
    out = nc.dram_tensor("out", [S], F32, kind="ExternalOutput")

    with tile.TileContext(nc) as tc:
        with (
            tc.tile_pool(name="wt", bufs=1) as wt_pool,
            tc.tile_pool(name="persist", bufs=1) as persist,
            tc.tile_pool(name="xc", bufs=2) as xc_pool,
            tc.tile_pool(name="dxc", bufs=2) as dxc_pool,
            tc.tile_pool(name="scratch", bufs=2) as scratch,
            tc.tile_pool(name="us", bufs=3) as us_pool,
            tc.tile_pool(name="ush", bufs=3) as ush_pool,
            tc.tile_pool(name="psum", bufs=3, space="PSUM") as psum_pool,
            tc.tile_pool(name="psum_misc", bufs=1, space="PSUM") as psum_misc,
        ):
            # ---- resident weights W.T, [d, e] layout, 8 partition chunks
            wt = []
            for k in range(NK):
                t = wt_pool.tile([128, D], BF16, tag=f"wt{k}")
                nc.gpsimd.dma_start(t[:], WT[k * 128 : (k + 1) * 128, :])
                wt.append(t)

            # ---- gate broadcast to [32, 1] via a tiny K=1 matmul
            g_sb = persist.tile([1, 1], F32, tag="g_sb")
            nc.sync.dma_start(g_sb[:], gate[:].rearrange("(a b) -> a b", a=1))
            ones32 = persist.tile([1, 32], F32, tag="ones32")
            nc.vector.memset(ones32[:], 1.0)
            g_ps = psum_misc.tile([32, 1], F32, tag="g_ps")
            nc.tensor.matmul(g_ps[:], lhsT=ones32[:], rhs=g_sb[:], start=True, stop=True)
            g32 = persist.tile([32, 1], F32, tag="g32")
            nc.scalar.activation(g32[:], g_ps[:], AF.Copy)
            a_col = persist.tile([32, 1], F32, tag="a_col")
            nc.scalar.mul(a_col[:], g32[:], 0.5 * ADJ_SCALE / (S - 2))
            b_col = persist.tile([32, 1], F32, tag="b_col")
            nc.scalar.mul(b_col[:], g32[:], -0.5 * ADJ_SCALE * 0.5)

            # ---- stats accumulators: s1 in cols [0,32), c in cols [32,64)
            stats = persist.tile([128, 64], F32, tag="stats")
            zrow = persist.tile([1, D], BF16, tag="zrow")
            nc.vector.memset(zrow[:], 0.0)

            # ---- main loop: stream xT, diff, matmul, fused reductions.
            # Partition-base rule (walrus): compute-engine APs must start at
            # partition 0/32/64/96 — in SBUF *and* PSUM. The u[i]*u[i+1]
            # cross-term therefore uses a DMA (partition-unrestricted) to build
            # a one-row-shifted bf16 copy of each U block, and a base-0 DVE
            # tensor_tensor_reduce against it.
            BPC = CHUNK // 128  # blocks per chunk (4)
            CW = CHUNK + 1  # loaded columns per chunk (1-col lookahead)
            prev = None  # (us_c, ush_c, c) of the previous chunk

            def emit_cross(us_c, ush_c, ci):
                # c[i] = sum_e u[i,e]*u[i+1,e] for the whole chunk: one DVE mul
                # + one 3D-AP reduce producing 4 stats columns.
                # (tensor_tensor_reduce / accum_out on DVE crash the exec unit
                # in this runtime; plain mul + tensor_reduce are fine.)
                cs = scratch.tile([128, BPC * D], BF16, tag="cs")
                nc.vector.tensor_mul(cs[:], us_c[:], ush_c[:])
                nc.vector.tensor_reduce(
                    stats[:, 32 + BPC * ci : 32 + BPC * (ci + 1)],
                    cs[:].rearrange("p (m e) -> p m e", e=D),
                    axis=mybir.AxisListType.X,
                    op=ALU.add,
                )

            for c in range(NCHUNK):
                last_chunk = c == NCHUNK - 1
                ncols = CHUNK if last_chunk else CW
                # one 2.1MB DMA per chunk: [128, 8, ncols] 3D access pattern
                xc = xc_pool.tile([128, NK * CW], F32, tag="xc")
                nc.sync.dma_start(
                    xc[:].rearrange("p (k j) -> p k j", k=NK)[:, :, 0:ncols],
                    xT[:, c * CHUNK : c * CHUNK + ncols].rearrange(
                        "(k p) j -> p k j", p=128
                    ),
                )
                # dx in bf16: one 3D DVE subtract per block (so block m's
                # matmuls never wait on later columns)
                dxc = dxc_pool.tile([128, NK * CHUNK], BF16, tag="dxc")
                x3 = xc[:].rearrange("p (k j) -> p k j", k=NK)
                d3 = dxc[:].rearrange("p (k j) -> p k j", k=NK)

                us_c = us_pool.tile([128, BPC * D], BF16, tag="us")
                ush_c = ush_pool.tile([128, BPC * D], BF16, tag="ush")
                for m in range(BPC):
                    g = c * BPC + m
                    lo = m * 128
                    hi = (m + 1) * 128
                    nd = hi - 1 if (last_chunk and m == BPC - 1) else hi
                    nc.vector.tensor_sub(
                        d3[:, :, lo:nd], x3[:, :, lo + 1 : nd + 1], x3[:, :, lo:nd]
                    )
                    if nd < hi:
                        nc.gpsimd.memset(d3[:, :, nd:hi], 0.0)
                    U = psum_pool.tile([128, D], F32, tag="U")
                    for n in range(2):
                        for k in range(NK):
                            nc.tensor.matmul(
                                U[:, n * 512 : (n + 1) * 512],
                                lhsT=dxc[:, k * CHUNK + lo : k * CHUNK + hi],
                                rhs=wt[k][:, n * 512 : (n + 1) * 512],
                                start=(k == 0),
                                stop=(k == NK - 1),
                            )
                    # s1[g*128+i] = sum_e U[i,e]^2  (ACT: square + row-accum)
                    sq = scratch.tile([128, D], F32, tag="sq")
                    nc.scalar.activation(
                        sq[:], U[:], AF.Square, accum_out=stats[:, g : g + 1]
                    )
                    # bf16 copy of U into the chunk-level buffer
                    nc.scalar.activation(us_c[:, m * D : (m + 1) * D], U[:], AF.Copy)

                # row-shifted copy via SWDGE DMAs (partition-unrestricted):
                # ush_c[i, m*D+e] = u[128*(4c+m) + i + 1, e]
                nc.gpsimd.dma_start(ush_c[0:127, :], us_c[1:128, :])
                nc.gpsimd.dma_start(
                    ush_c[127:128, 0 : (BPC - 1) * D], us_c[0:1, D : BPC * D]
                )
                if prev is not None:
                    pus_c, push_c, pc_ = prev
                    nc.gpsimd.dma_start(
                        push_c[127:128, (BPC - 1) * D : BPC * D], us_c[0:1, 0:D]
                    )
                    emit_cross(pus_c, push_c, pc_)
                prev = (us_c, ush_c, c)
            # final chunk: u[4096] does not exist -> zero row, c[4095] unused
            pus_c, push_c, pc_ = prev
            nc.gpsimd.dma_start(push_c[127:128, (BPC - 1) * D : BPC * D], zrow[:])
            emit_cross(pus_c, push_c, pc_)

            # ---- transpose stats [128, 64] -> [64, 128]: rows 0..31 = s1_t,
            #      rows 32..63 = c_t, column j = within-block index i
            ident = persist.tile([128, 128], F32, tag="ident")
            make_identity(nc, ident[:])
            st_ps = psum_misc.tile([64, 128], F32, tag="st_ps")
            nc.tensor.transpose(st_ps[:], stats[:], ident[:])
            s1_t = persist.tile([32, 128], F32, tag="s1_t")
            nc.scalar.activation(s1_t[:], st_ps[0:32, :], AF.Copy)
            c_t = persist.tile([32, 128], F32, tag="c_t")
            nc.scalar.activation(c_t[:], st_ps[32:64, :], AF.Copy)

            # ---- s1 shifted by one flat position: s1n[m, j] = s1[128m + j + 1]
            # main part is a free-dim shift; seam column 127 needs s1[128(m+1)]
            # = stats[0, m+1], partition-scattered via a tiny DMA.
            s1n = persist.tile([32, 128], F32, tag="s1n")
            nc.vector.tensor_copy(s1n[:, 0:127], s1_t[:, 1:128])
            row32 = persist.tile([1, 32], F32, tag="row32")
            nc.vector.tensor_copy(row32[0:1, 0:31], stats[0:1, 1:32])
            nc.vector.memset(row32[0:1, 31:32], 0.0)
            nc.sync.dma_start(s1n[0:32, 127:128], row32[0:1, 0:32])

            # s2 = s1 + s1n + 2c
            s2_t = persist.tile([32, 128], F32, tag="s2_t")
            nc.vector.tensor_add(s2_t[:], s1_t[:], s1n[:])
            c2_t = persist.tile([32, 128], F32, tag="c2_t")
            nc.vector.tensor_scalar_mul(c2_t[:], c_t[:], 2.0)
            nc.vector.tensor_add(s2_t[:], s2_t[:], c2_t[:])

            # d1[i], d1[i+1], d2[i]
            d1_t = persist.tile([32, 128], F32, tag="d1_t")
            nc.scalar.activation(d1_t[:], s1_t[:], AF.Sqrt)
            d1n = persist.tile([32, 128], F32, tag="d1n")
            nc.scalar.activation(d1n[:], s1n[:], AF.Sqrt)
            d2_t = persist.tile([32, 128], F32, tag="d2_t")
            nc.scalar.activation(d2_t[:], s2_t[:], AF.Sqrt)

            # path[i] = d1[i] + d1[i+1] (no seams: both operands flat-aligned)
            path = persist.tile([32, 128], F32, tag="path")
            nc.vector.tensor_add(path[:], d1_t[:], d1n[:])

            # score = relu(1 - (path - d2) / max(d2, eps))
            denom = persist.tile([32, 128], F32, tag="denom")
            nc.vector.tensor_scalar_max(denom[:], d2_t[:], EPS)
            rec = persist.tile([32, 128], F32, tag="rec")
            nc.vector.reciprocal(rec[:], denom[:])
            num = persist.tile([32, 128], F32, tag="num")
            nc.vector.tensor_sub(num[:], path[:], d2_t[:])
            ratio = persist.tile([32, 128], F32, tag="ratio")
            nc.vector.tensor_mul(ratio[:], num[:], rec[:])
            score = persist.tile([32, 128], F32, tag="score")
            nc.scalar.activation(score[:], ratio[:], AF.Relu, scale=-1.0, bias=1.0)

            # adj[i] = a*score[i] + b, shipped to out[i+1] via DMA addressing;
            # boundary cells out[0], out[4095] get the bare b value.
            adj_t = persist.tile([32, 128], F32, tag="adj_t")
            nc.vector.tensor_scalar(
                out=adj_t[:],
                in0=score[:],
                scalar1=a_col[:],
                scalar2=b_col[:],
                op0=ALU.mult,
                op1=ALU.add,
            )
            bb = persist.tile([1, 2], F32, tag="bb")
            nc.scalar.activation(bb[0:1, 0:1], b_col[0:1, :], AF.Copy)
            nc.scalar.activation(bb[0:1, 1:2], b_col[0:1, :], AF.Copy)

            # out[1 : 3969] <- adj flat [0 : 3968)
            nc.sync.dma_start(
                out[1:3969].rearrange("(p f) -> p f", f=128), adj_t[0:31, :]
            )
            # out[3969 : 4095] <- adj flat [3968 : 4094)
            nc.sync.dma_start(
                out[3969:4095].rearrange("(p f) -> p f", p=1), adj_t[31:32, 0:126]
            )
            nc.sync.dma_start(out[0:1].rearrange("(p f) -> p f", p=1), bb[0:1, 0:1])
            nc.sync.dma_start(out[4095:4096].rearrange("(p f) -> p f", p=1), bb[0:1, 1:2])

    nc.compile()
    return nc


_NC_CACHE = None


def kernel(x, W, b, gate):
    global _NC_CACHE
    x = np.asarray(x, dtype=np.float32)
    W = np.asarray(W, dtype=np.float32)
    gate = np.asarray(gate, dtype=np.float32)

    if _NC_CACHE is None:
        _NC_CACHE = build_nc()
    nc = _NC_CACHE

    WT_np = np.ascontiguousarray(W.T)
    in_maps = [
        {
            "xT": np.ascontiguousarray(x[i].T),
            "WT": WT_np,
            "gate": gate,
        }
        for i in range(B)
    ]
    res = run_bass_kernel_spmd(nc, in_maps, core_ids=list(range(B)))
    return np.stack([res.results[i]["out"] for i in range(B)]).astype(np.float32)


if __name__ == "__main__":
    # quick smoke: build only
    nc = build_nc()
    print("built ok")
